# revision 4
# baseline (speedup 1.0000x reference)
"""Trainium2 kernel for nn_Graph_41609643163904.

The reference op is a sequential per-cell scatter sweep over a 48x48 grid:
for x in 2..45, y in 2..45 (x outer): read center v, zero it, add v*W[y,x]
to the 5x5 neighborhood.  Every step is linear in the grid, so the whole
sweep is one fixed linear operator M (2304x2304) depending only on the
weights.  We build M on the host (cheap: 1936 rank-1 row updates), then the
device work is a batched matmul out = in @ M^T, data-parallel over the
8192-sample batch across 8 NeuronCores (1024 samples/core), zero comm.

Key device-side tricks:
  * float32r matmul: full fp32 precision at 1 cycle/row (bf16-rate) on the
    PE array for moving dim >= 256.
  * x-major re-flattening of the grid exposes the sweep's causal cone as
    block sparsity: influence travels at most 2 columns leftward, so in
    x-major order M' is block-banded.  Per 384-wide j-tile only a prefix
    of k-tiles is nonzero: [4,7,10,13,16,18] of 18 -> 63% of the dense
    work and of the M DMA traffic.
  * M' blocks stay resident in SBUF (13.1 MB); batch tiles stream through.
"""

import os

import numpy as np

SIZE = 48
D = 2
K = 5
N = SIZE * SIZE          # 2304
B = 8192
NCORES = 8
BS = B // NCORES         # 1024 samples per core

P = 128
NK = N // P              # 18 k-tiles
JW = 384                 # j-tile width (one PSUM bank holds 512 fp32)
NJ = N // JW             # 6 j-tiles
MB = BS // P             # 8 m-tiles per core

# Structural nonzero k-tile prefix per j-tile (x-major layout).  A cell's
# influence reaches at most 2 grid-columns to the left of its own column,
# so M'[j,k] == 0 whenever jx < kx - 2.  j-tile t covers jx <= 8t+7, hence
# k < 48*(8t+10) -> ceil to 128: these prefixes are valid for ANY weights.
KPREF = tuple(min(NK, -(-(SIZE * (8 * t + 10)) // P)) for t in range(NJ))
NBLK = sum(KPREF)        # 68


def _build_M(weights: np.ndarray) -> np.ndarray:
    """Compose the 1936 per-cell updates into one (N, N) operator, fp64."""
    M = np.eye(N, dtype=np.float64)
    w = weights.astype(np.float64)
    for x in range(D, SIZE - D):
        for y in range(D, SIZE - D):
            c = y * SIZE + x
            wc = w[y, x]
            rc = M[c].copy()
            for dy in range(-D, D + 1):
                r0 = c + dy * SIZE - D
                wrow = wc[dy + D]
                if dy == 0:
                    M[r0:r0 + D] += np.outer(wrow[:D], rc)
                    M[r0 + D + 1:r0 + K] += np.outer(wrow[D + 1:], rc)
                else:
                    M[r0:r0 + K] += np.outer(wrow, rc)
            M[c] = wc[D, D] * rc
    return M


def _build_device_kernel():
    import concourse.mybir as mybir
    from concourse import bacc
    from concourse.tile import TileContext

    f32 = mybir.dt.float32
    f32r = mybir.dt.float32r

    nc = bacc.Bacc()
    xT = nc.dram_tensor("xT", [N, BS], f32r, kind="ExternalInput")
    mt = nc.dram_tensor("mt", [NBLK * P, JW], f32r, kind="ExternalInput")
    out = nc.dram_tensor("out", [BS, N], f32, kind="ExternalOutput")

    xT_r = xT.rearrange("(k p) m -> k p m", p=P)
    mt_r = mt.rearrange("(b p) c -> b p c", p=P)

    with TileContext(nc) as tc:
        with (
            tc.tile_pool(name="mpool", bufs=1) as mpool,
            tc.tile_pool(name="xpool", bufs=2) as xpool,
            tc.tile_pool(name="opool", bufs=2) as opool,
            tc.tile_pool(name="pspool", bufs=1, space="PSUM") as pspool,
        ):
            # Load all M' blocks resident, ordered k-major so the first
            # m-tile's accumulation can start as soon as possible.
            order = [(t, k) for t in range(NJ) for k in range(KPREF[t])]
            block_idx = {tk: i for i, tk in enumerate(order)}
            mtiles = {}
            for t, k in sorted(order, key=lambda tk: (tk[1], tk[0])):
                mm = mpool.tile([P, JW], f32r, tag=f"m{t}_{k}", name=f"m{t}_{k}")
                nc.sync.dma_start(out=mm[:], in_=mt_r[block_idx[(t, k)]])
                mtiles[(t, k)] = mm

            for m in range(MB):
                xt = xpool.tile([P, NK * P], f32r, tag="x", name=f"x{m}")
                for k in range(NK):
                    nc.sync.dma_start(
                        out=xt[:, k * P:(k + 1) * P],
                        in_=xT_r[k, :, m * P:(m + 1) * P],
                    )
                ot = opool.tile([P, N], f32, tag="o", name=f"o{m}")
                ps = {
                    t: pspool.tile([P, JW], f32, tag=f"ps{t}", name=f"ps{m}_{t}")
                    for t in range(NJ)
                }
                for k in range(NK):
                    for t in range(NJ):
                        if k >= KPREF[t]:
                            continue
                        nc.tensor.matmul(
                            ps[t][:],
                            lhsT=xt[:, k * P:(k + 1) * P],
                            rhs=mtiles[(t, k)][:],
                            start=(k == 0),
                            stop=(k == KPREF[t] - 1),
                        )
                        if k == KPREF[t] - 1:
                            nc.vector.tensor_copy(
                                ot[:, t * JW:(t + 1) * JW], ps[t][:]
                            )
                            nc.sync.dma_start(
                                out=out[m * P:(m + 1) * P, t * JW:(t + 1) * JW],
                                in_=ot[:, t * JW:(t + 1) * JW],
                            )
    if not nc.is_finalized():
        nc.finalize()
    return nc


_XMAJOR_IDX = None


def _xmajor_idx():
    global _XMAJOR_IDX
    if _XMAJOR_IDX is None:
        n = np.arange(N)
        _XMAJOR_IDX = (n % SIZE) * SIZE + n // SIZE
    return _XMAJOR_IDX


def kernel(inputs: np.ndarray, weights: np.ndarray) -> np.ndarray:
    from concourse.bass_utils import run_bass_kernel_spmd

    inputs = np.ascontiguousarray(inputs, dtype=np.float32)
    weights = np.ascontiguousarray(weights, dtype=np.float32)

    # Host: build the composed operator and permute to x-major layout.
    M = _build_M(weights)
    idx = _xmajor_idx()
    MTp = np.ascontiguousarray(M[np.ix_(idx, idx)].T.astype(np.float32))

    blocks = [
        MTp[k * P:(k + 1) * P, t * JW:(t + 1) * JW]
        for t in range(NJ)
        for k in range(KPREF[t])
    ]
    mt_packed = np.ascontiguousarray(np.concatenate(blocks, axis=0))

    # x-major per-sample flatten, then transpose so k is the leading dim.
    xP = inputs.reshape(B, SIZE, SIZE).transpose(0, 2, 1).reshape(B, N)

    nc = _build_device_kernel()
    in_maps = [
        {
            "xT": np.ascontiguousarray(xP[c * BS:(c + 1) * BS].T),
            "mt": mt_packed,
        }
        for c in range(NCORES)
    ]
    trace = bool(int(os.environ.get("KERNEL_TRACE", "0")))
    res = run_bass_kernel_spmd(
        nc, in_maps, core_ids=list(range(NCORES)), trace=trace
    )
    if trace and res.exec_time_ns is not None:
        print(f"HW exec time: {res.exec_time_ns} ns")
        if res.instructions_and_trace is not None:
            print(f"trace: {res.instructions_and_trace[1]}")

    outP = np.concatenate([res.results[c]["out"] for c in range(NCORES)], axis=0)
    return np.ascontiguousarray(
        outP.reshape(B, SIZE, SIZE).transpose(0, 2, 1).reshape(B, N)
    )
